# revision 2
# baseline (speedup 1.0000x reference)
"""DecoderRNN Trainium2 kernel v2 (8 NeuronCores, zero collectives).

Sharding: pure data-parallel over batch (16 rows/core). Each core runs the
full attention+GRU recurrence in transposed layout ([feature-part, batch-free])
and the FULL-vocab output projection + log-softmax locally (no AllGather /
AllReduce). fp8e4m3 DoubleRow matmuls for the attention MLP and the output
projection; ACT engine runs tanh-only during the recurrence (sigmoid via
tanh(x/2), softmax exp via poly on GpSimd) to avoid activation-table reloads.

Self-contained: hardcodes all shapes from the problem spec.
"""
from contextlib import ExitStack

import numpy as np
import ml_dtypes

import concourse.bacc as bacc
import concourse.bass as bass
import concourse.tile as tile
from concourse import mybir
from concourse.bass import AP
from concourse.masks import make_identity

F32 = mybir.dt.float32
BF16 = mybir.dt.bfloat16
F8 = mybir.dt.float8e4
I32 = mybir.dt.int32
AF = mybir.ActivationFunctionType
DR = mybir.MatmulPerfMode.DoubleRow

# problem constants
B, L, H, V, WORD, T = 128, 64, 512, 32000, 512, 32
NC = 8
BL = B // NC          # 16 batch rows per core
NR = BL * L           # 1024 attention rows (l-major: row = l*BL + b)
RK = NR // 128        # 8 row chunks
HK = H // 128         # 4 h chunks
G3 = 3 * H            # 1536
GM = G3 // 128        # 12 gate chunks
TS = T - 1            # 31 steps
TB = TS * BL          # 496 (t-major cols: col = t*BL + b)
BS = 8                # steps per phase-2 block
VT = 512              # vocab tile
TCH = 500             # pass-A exp chunk cols

EXP_C = [1.0, 1.0, 0.5, 1.0 / 6, 1.0 / 24, 1.0 / 120, 1.0 / 720]


def _mm(nc, out, lhsT, rhs, start, stop, pm=None):
    nc.tensor.matmul(out, lhsT, rhs, start=start, stop=stop, perf_mode=pm)


def _bcast(ap, dims):
    """Rebuild AP with explicit [stride, count] dim list."""
    return AP(tensor=ap.tensor, offset=ap.offset, ap=dims)


def build_program(t_steps=TS, use_att_bias=False, use_gru_bias=False,
                  use_out_bias=False):
    nc = bacc.Bacc("TRN2", target_bir_lowering=False, debug=False,
                   num_devices=NC)
    # phase-2 blocks
    blocks = []
    t0 = 0
    while t0 < t_steps:
        blocks.append((t0, min(BS, t_steps - t0)))
        t0 += BS
    NB = len(blocks)
    n_vt = (V + VT - 1) // VT          # 63 (62x512 + 256)
    n_ch = (V + TCH - 1) // TCH        # 32
    tb = t_steps * BL

    def din(name, shape, dt=F32):
        return nc.dram_tensor(name, shape, dt, kind="ExternalInput")

    enc_sb_in = din("enc_sb", [RK, 128, H + 1], BF16)
    encT_in = din("encT", [HK, 128, NR], BF16)
    w1eT_in = din("w1eT", [HK, 128, H], BF16)
    w1hT_in = din("w1hT", [HK, 128, H], BF16)
    w2T8_in = din("w2T8", [HK, 128, H], F8)
    w3T8_in = din("w3T8", [HK, 128, H], F8)
    vT8_in = din("vT8", [HK, 128, 1], F8)
    whhT_in = din("whhT", [HK, 128, G3], BF16)
    wihcT_in = din("wihcT", [HK, 128, G3], BF16)
    wiheT_in = din("wiheT", [HK, 128, G3], BF16)
    embT_in = din("embT", [HK, 128, tb], BF16)
    outWT8_in = din("outWT8", [HK, 128, V], F8)
    sel_in = din("sel", [128, BL])
    hidT0_in = din("hidT0", [HK, 128, BL])
    if use_att_bias:
        b1_in = din("b1", [1, H])
        b2_in = din("b2", [128, HK])
        b3_in = din("b3", [128, HK])
    if use_gru_bias:
        bih_in = din("bih", [1, G3])
        brz_in = din("brz", [128, 8])      # (bih+bhh) r,z parts, T-chunked
        bihn_in = din("bihn", [128, 4])
        bhhn_in = din("bhhn", [128, 4])
    if use_out_bias:
        outb_in = din("outb", [1, V], BF16)
    out_lp = nc.dram_tensor("out_lp", [NB, 128, V], F32,
                            kind="ExternalOutput")

    with tile.TileContext(nc) as tc, ExitStack() as top:
        dram = top.enter_context(tc.tile_pool(name="dram", bufs=1, space="DRAM"))
        giemb_d = dram.tile([GM, 128, tb], BF16)
        lstage_d = dram.tile([NB, 128, V], BF16)

        per = top.enter_context(tc.tile_pool(name="per", bufs=1))
        ident = per.tile([16, 16], F32)
        make_identity(nc, ident[:])
        sel_sb = per.tile([128, BL], F32)
        nc.sync.dma_start(sel_sb[:], sel_in.ap())
        enc_sb = per.tile([128, RK, H + 1], BF16)
        nc.sync.dma_start(enc_sb[:], enc_sb_in.ap().rearrange("k p h -> p k h"))
        w1hT_sb = per.tile([128, HK, H], BF16)
        nc.sync.dma_start(w1hT_sb[:], w1hT_in.ap().rearrange("k p h -> p k h"))
        w2T8_sb = per.tile([128, HK, H], F8)
        nc.sync.dma_start(w2T8_sb[:], w2T8_in.ap().rearrange("k p h -> p k h"))
        w3T8_sb = per.tile([128, HK, H], F8)
        nc.sync.dma_start(w3T8_sb[:], w3T8_in.ap().rearrange("k p h -> p k h"))
        vT8_sb = per.tile([128, HK], F8)
        nc.sync.dma_start(vT8_sb[:], vT8_in.ap().rearrange("k p one -> p (k one)"))
        whhT_sb = per.tile([128, HK, G3], BF16)
        nc.sync.dma_start(whhT_sb[:], whhT_in.ap().rearrange("k p g -> p k g"))
        wihcT_sb = per.tile([128, HK, G3], BF16)
        nc.sync.dma_start(wihcT_sb[:], wihcT_in.ap().rearrange("k p g -> p k g"))
        encprojT = per.tile([128, HK, NR], BF16)
        if use_att_bias:
            b2_sb = per.tile([128, HK], F32)
            nc.sync.dma_start(b2_sb[:], b2_in.ap())
            b3_sb = per.tile([128, HK], F32)
            nc.sync.dma_start(b3_sb[:], b3_in.ap())
        if use_gru_bias:
            brz_sb = per.tile([128, 8], F32)
            nc.sync.dma_start(brz_sb[:], brz_in.ap())
            bihn_sb = per.tile([128, 4], F32)
            nc.sync.dma_start(bihn_sb[:], bihn_in.ap())
            bhhn_sb = per.tile([128, 4], F32)
            nc.sync.dma_start(bhhn_sb[:], bhhn_in.ap())

        # ---------------- phase 0: encproj + gi_emb precompute --------------
        with ExitStack() as ph0:
            p0 = ph0.enter_context(tc.tile_pool(name="p0", bufs=1))
            p0psum = ph0.enter_context(tc.tile_pool(name="p0ps", bufs=2,
                                                    space="PSUM"))
            encT_sb = p0.tile([128, HK, NR], BF16)
            nc.sync.dma_start(encT_sb[:], encT_in.ap().rearrange("k p r -> p k r"))
            w1eT_sb = p0.tile([128, HK, H], BF16)
            nc.sync.dma_start(w1eT_sb[:], w1eT_in.ap().rearrange("k p h -> p k h"))
            wiheT_sb = p0.tile([128, HK, G3], BF16)
            nc.sync.dma_start(wiheT_sb[:], wiheT_in.ap().rearrange("k p g -> p k g"))
            embT_sb = p0.tile([128, HK, tb], BF16)
            nc.sync.dma_start(embT_sb[:], embT_in.ap().rearrange("k p c -> p k c"))
            if use_att_bias:
                b1_sb = p0.tile([1, H], F32)
                nc.sync.dma_start(b1_sb[:], b1_in.ap())
            if use_gru_bias:
                bih_sb = p0.tile([1, G3], F32)
                nc.sync.dma_start(bih_sb[:], bih_in.ap())
            if use_att_bias or use_gru_bias:
                ones_nr = p0.tile([1, 512], F32)
                nc.gpsimd.memset(ones_nr[:], 1.0)

            # encprojT[p=h'%128, m, row] = W1e @ enc^T (+ b1)
            for m in range(HK):
                for half in range(2):
                    sl = slice(half * 512, (half + 1) * 512)
                    pep = p0psum.tile([128, 512], F32, tag="p0ps")
                    for k in range(HK):
                        _mm(nc, pep[:], w1eT_sb[:, k, m * 128:(m + 1) * 128],
                            encT_sb[:, k, sl], k == 0,
                            (not use_att_bias) and k == HK - 1)
                    if use_att_bias:
                        _mm(nc, pep[:], b1_sb[:, m * 128:(m + 1) * 128],
                            ones_nr[:], False, True)
                    nc.scalar.activation(out=encprojT[:, m, sl], in_=pep[:],
                                         func=AF.Copy)
            # gi_embT[p=g%128, m, col=(t,b)] = Wih_e @ emb^T (+ bih)
            p0s = ph0.enter_context(tc.tile_pool(name="p0s", bufs=2))
            for m in range(GM):
                pge = p0psum.tile([128, tb], F32, tag="p0ps")
                for k in range(HK):
                    _mm(nc, pge[:], wiheT_sb[:, k, m * 128:(m + 1) * 128],
                        embT_sb[:, k, :], k == 0,
                        (not use_gru_bias) and k == HK - 1)
                if use_gru_bias:
                    _mm(nc, pge[:], bih_sb[:, m * 128:(m + 1) * 128],
                        ones_nr[:, 0:tb], False, True)
                ge_sb = p0s.tile([128, tb], BF16, tag="ge_sb")
                nc.scalar.activation(out=ge_sb[:], in_=pge[:], func=AF.Copy)
                nc.sync.dma_start(giemb_d[m], ge_sb[:])

        # big fp8 weight load after phase-0 SBUF is released (spread queues)
        outWT8_sb = per.tile([128, HK, V], F8)
        for k in range(HK):
            nc.gpsimd.dma_start(outWT8_sb[:, k, :], outWT8_in.ap()[k])
        nlzs = per.tile([128, NB], F32)
        if use_out_bias:
            outb_sb = per.tile([1, V], BF16)
            nc.sync.dma_start(outb_sb[:], outb_in.ap())
            onesb = per.tile([1, 128], BF16)
            nc.gpsimd.memset(onesb[:], 1.0)

        # ---------------- phase 1 + overlapped phase 2 ----------------------
        ph1 = top.enter_context(ExitStack())
        hidp = ph1.enter_context(tc.tile_pool(name="hidp", bufs=2))
        gw = ph1.enter_context(tc.tile_pool(name="gw", bufs=1))
        wk = ph1.enter_context(tc.tile_pool(name="wk", bufs=2))
        gep = ph1.enter_context(tc.tile_pool(name="gep", bufs=2))
        htp = ph1.enter_context(tc.tile_pool(name="htp", bufs=2))
        lst = ph1.enter_context(tc.tile_pool(name="lst", bufs=2))
        tpa = ph1.enter_context(tc.tile_pool(name="tpa", bufs=2))
        tpb = ph1.enter_context(tc.tile_pool(name="tpb", bufs=2))
        TB2 = 500
        n_c2 = (V + TB2 - 1) // TB2
        # PSUM: pd 2x2 + pp 1 + pg 1 (gh/gi/e/Z) + pm 1 + pcu 1 = 8 banks
        pd = ph1.enter_context(tc.tile_pool(name="pd", bufs=2, space="PSUM"))
        pp = ph1.enter_context(tc.tile_pool(name="pp", bufs=1, space="PSUM"))
        pgp = ph1.enter_context(tc.tile_pool(name="pg", bufs=1, space="PSUM"))
        pmp = ph1.enter_context(tc.tile_pool(name="pm", bufs=1, space="PSUM"))

        hidf = hidp.tile([128, HK, BL], F32, tag="hidf")
        nc.sync.dma_start(hidf[:], hidT0_in.ap().rearrange("k p b -> p k b"))
        hidT = hidp.tile([128, HK, BL], BF16, tag="hidT")
        nc.vector.tensor_copy(hidT[:], hidf[:])

        t2blk = {}
        for i, (bt0, bs) in enumerate(blocks):
            for tt in range(bs):
                t2blk[bt0 + tt] = (i, tt)
        hT_blk = None
        for t in range(t_steps):
            nb, tt = t2blk[t]
            if tt == 0:
                hT_blk = htp.tile([128, HK, BS, BL], F8, tag="hT")

            ge = gep.tile([128, GM, BL], BF16, tag="ge")
            nc.sync.dma_start(
                ge[:], giemb_d[:, :, t * BL:(t + 1) * BL]
                .rearrange("m p b -> p m b"))

            # hidproj (pm slot)
            php = pmp.tile([128, HK, BL], F32, tag="pm")
            for m in range(HK):
                for k in range(HK):
                    _mm(nc, php[:, m, :], w1hT_sb[:, k, m * 128:(m + 1) * 128],
                        hidT[:, k, :], k == 0, k == HK - 1)
            hp_sb = gw.tile([128, HK, BL], BF16, tag="hp_sb")
            nc.scalar.activation(out=hp_sb[:], in_=php[:], func=AF.Copy)

            # psum: [0:8] = gh_rz + gi_rz (merged groups), [8:12] = gh_n,
            # [12:16] = gi_n
            pg = pgp.tile([128, 16, BL], F32, tag="pg")
            for m in range(8, GM):      # gh_n early (PE idle at step start)
                for k in range(HK):
                    _mm(nc, pg[:, m, :], whhT_sb[:, k, m * 128:(m + 1) * 128],
                        hidT[:, k, :], k == 0, k == HK - 1)
            pgh = pg[:, 0:GM, :]

            # attention MLP over row-halves
            a13 = gw.tile([128, HK, NR], F8, tag="a13")   # a1 then a3 in place
            a2T8 = gw.tile([128, HK, NR], F8, tag="a2T8")
            pe = pmp.tile([128, RK], F32, tag="pm")
            e_sb = gw.tile([128, RK], F32, tag="e_sb")
            pt1 = gw.tile([128, RK], F32, tag="pt1")
            mask_sb = gw.tile([128, RK, BL], BF16, tag="mask")
            pcu = pmp.tile([16, 512], F32, tag="pm2")
            for hf in range(2):
                sl = slice(hf * 512, (hf + 1) * 512)
                a1pre = wk.tile([128, HK, 512], BF16, tag="a1pre")
                hb = _bcast(hp_sb[:], [hp_sb[:].ap[0], [BL, HK], [0, 32], [1, BL]])
                nc.vector.tensor_add(a1pre[:], encprojT[:, :, sl], hb)
                nc.scalar.activation(out=a13[:, :, sl], in_=a1pre[:],
                                     func=AF.Tanh)
                for (src, dst, wT, bsb) in ((a13, a2T8, w2T8_sb, "b2"),
                                            (a2T8, a13, w3T8_sb, "b3")):
                    for mp in range(2):      # m-pairs -> one 2-bank psum tile
                        pdt = pd.tile([128, 2, 512], F32, tag="pd")
                        for mi in range(2):
                            m = 2 * mp + mi
                            for kk in range(2):
                                _mm(nc, pdt[:, mi, :],
                                    wT[:, 2 * kk:2 * kk + 2,
                                       m * 128:(m + 1) * 128],
                                    src[:, 2 * kk:2 * kk + 2, sl],
                                    kk == 0, kk == 1, pm=DR)
                        if use_att_bias:
                            bb = b2_sb if bsb == "b2" else b3_sb
                            for mi in range(2):
                                m = 2 * mp + mi
                                nc.scalar.activation(out=dst[:, m, sl],
                                                     in_=pdt[:, mi, :],
                                                     func=AF.Tanh,
                                                     bias=bb[:, m:m + 1])
                        else:
                            nc.scalar.activation(
                                out=dst[:, 2 * mp:2 * mp + 2, sl],
                                in_=pdt[:], func=AF.Tanh)
                # e rows for this half (a13 now holds a3)
                rsl = slice(hf * 4, hf * 4 + 4)
                for r in range(hf * 4, hf * 4 + 4):
                    for k in range(HK):
                        _mm(nc, pe[:, r:r + 1],
                            a13[:, k, r * 128:(r + 1) * 128],
                            vT8_sb[:, k:k + 1], k == 0, k == HK - 1)
                # exp(e) poly-6 on DVE (keeps ACT tanh-only)
                nc.vector.tensor_copy(e_sb[:, rsl], pe[:, rsl])
                nc.gpsimd.tensor_scalar(pt1[:, rsl], e_sb[:, rsl], EXP_C[6],
                                        EXP_C[5], mybir.AluOpType.mult,
                                        mybir.AluOpType.add)
                for dg in (4, 3, 2, 1, 0):
                    nc.gpsimd.tensor_mul(pt1[:, rsl], pt1[:, rsl], e_sb[:, rsl])
                    nc.gpsimd.tensor_scalar_add(pt1[:, rsl], pt1[:, rsl],
                                                EXP_C[dg])
                ea = _bcast(pt1[:, rsl], [pt1[:].ap[0], [1, 4], [0, BL]])
                sa = _bcast(sel_sb[:], [sel_sb[:].ap[0], [0, 4], [1, BL]])
                nc.vector.tensor_mul(mask_sb[:, rsl, :], ea, sa)
                for k in range(hf * 4, hf * 4 + 4):
                    _mm(nc, pcu[:], mask_sb[:, k, :], enc_sb[:, k, 0:H],
                        k == 0, k == RK - 1)

            pz = pmp.tile([16, 1], F32, tag="pm")
            for k in range(RK):
                _mm(nc, pz[:], mask_sb[:, k, :], enc_sb[:, k, H:H + 1],
                    k == 0, k == RK - 1)
            ctxu_sb = gw.tile([16, 512], BF16, tag="ctxu")
            nc.scalar.activation(out=ctxu_sb[:], in_=pcu[:], func=AF.Copy)
            rcpZ = gw.tile([16, 1], F32, tag="rcpZ")
            nc.vector.reciprocal(rcpZ[:], pz[:])
            diag = gw.tile([16, 16], BF16, tag="diag")
            nc.vector.tensor_scalar_mul(diag[:], ident[:], rcpZ[:])
            pct = pmp.tile([128, HK, BL], F32, tag="pm")
            for m in range(HK):
                _mm(nc, pct[:, m, :], ctxu_sb[:, m * 128:(m + 1) * 128],
                    diag[:], True, True)
            ctxT_sb = gw.tile([128, HK, BL], BF16, tag="ctxT")
            nc.vector.tensor_copy(ctxT_sb[:], pct[:])

            # gi: rz parts continue gh_rz accumulation groups; n part separate
            for m in range(8):
                for k in range(HK):
                    _mm(nc, pg[:, m, :], whhT_sb[:, k, m * 128:(m + 1) * 128],
                        hidT[:, k, :], k == 0, False)
                for k in range(HK):
                    _mm(nc, pg[:, m, :], wihcT_sb[:, k, m * 128:(m + 1) * 128],
                        ctxT_sb[:, k, :], False, k == HK - 1)
            for m in range(4):
                for k in range(HK):
                    _mm(nc, pg[:, 12 + m, :],
                        wihcT_sb[:, k, (8 + m) * 128:(9 + m) * 128],
                        ctxT_sb[:, k, :], k == 0, k == HK - 1)
            pgi = _bcast(pg[:, 12:16, :], None)  # placeholder, unused
            # gates: s = tanh(x/2); sigmoid(x) = (s+1)/2
            rzs = gw.tile([128, 8, BL], F32, tag="rzs")
            nc.vector.tensor_add(rzs[:], pg[:, 0:8, :], ge[:, 0:8, :])
            if use_gru_bias:
                for m in range(8):
                    nc.vector.tensor_scalar_add(rzs[:, m, :], rzs[:, m, :],
                                                brz_sb[:, m:m + 1])
            srz = gw.tile([128, 8, BL], F32, tag="srz")
            nc.scalar.activation(out=srz[:], in_=rzs[:], func=AF.Tanh, scale=0.5)
            u1 = gw.tile([128, 4, BL], F32, tag="u1")
            nc.vector.tensor_add(u1[:], pgi[:, 8:12, :], ge[:, 8:12, :])
            if use_gru_bias:
                for m in range(4):
                    nc.vector.tensor_scalar_add(u1[:, m, :], u1[:, m, :],
                                                bihn_sb[:, m:m + 1])
            u2 = gw.tile([128, 4, BL], F32, tag="u2")
            if use_gru_bias:
                ghnb = gw.tile([128, 4, BL], F32, tag="ghnb")
                for m in range(4):
                    nc.vector.tensor_scalar_add(ghnb[:, m, :], pgh[:, 8 + m, :],
                                                bhhn_sb[:, m:m + 1])
                nc.vector.tensor_mul(u2[:], srz[:, 0:4, :], ghnb[:])
                nc.vector.tensor_add(u2[:], u2[:], ghnb[:])
            else:
                nc.vector.tensor_mul(u2[:], srz[:, 0:4, :], pgh[:, 8:12, :])
                nc.vector.tensor_add(u2[:], u2[:], pgh[:, 8:12, :])
            nc.vector.tensor_scalar_mul(u2[:], u2[:], 0.5)
            nc.vector.tensor_add(u1[:], u1[:], u2[:])
            n_t = gw.tile([128, 4, BL], F32, tag="n_t")
            nc.scalar.activation(out=n_t[:], in_=u1[:], func=AF.Tanh)
            d_t = gw.tile([128, 4, BL], F32, tag="d_t")
            nc.vector.tensor_sub(d_t[:], hidf[:], n_t[:])
            v1 = gw.tile([128, 4, BL], F32, tag="v1")
            nc.vector.tensor_mul(v1[:], srz[:, 4:8, :], d_t[:])
            nc.vector.tensor_add(v1[:], v1[:], d_t[:])
            nc.vector.tensor_scalar_mul(v1[:], v1[:], 0.5)
            hidf = hidp.tile([128, HK, BL], F32, tag="hidf")
            nc.vector.tensor_add(hidf[:], n_t[:], v1[:])
            hidT = hidp.tile([128, HK, BL], BF16, tag="hidT")
            nc.vector.tensor_copy(hidT[:], hidf[:])
            nc.vector.tensor_copy(hT_blk[:, :, tt, :], hidf[:])

            # -------- phase 2 pass A for completed block ----------
            if tt == blocks[nb][1] - 1:
                bs = tt + 1
                rows = bs * BL
                acc = tpa.tile([128, n_vt], F32, tag="acc")
                exs = tpa.tile([128, VT], BF16, tag="exs")
                for j in range(n_vt):
                    c0 = j * VT
                    w = min(VT, V - c0)
                    ppt = pp.tile([128, VT], F32, tag="pp")
                    for kk in range(2):
                        _mm(nc, ppt[0:rows, 0:w],
                            hT_blk[:, 2 * kk:2 * kk + 2, 0:bs, :],
                            outWT8_sb[:, 2 * kk:2 * kk + 2, c0:c0 + w],
                            kk == 0, kk == 1 and not use_out_bias, pm=DR)
                    if use_out_bias:
                        _mm(nc, ppt[0:rows, 0:w], onesb[:, 0:rows],
                            outb_sb[:, c0:c0 + w], False, True)
                    lsg = lst.tile([128, VT], BF16, tag="lsg")
                    nc.vector.tensor_copy(lsg[0:rows, 0:w], ppt[0:rows, 0:w])
                    nc.sync.dma_start(lstage_d[nb, 0:rows, c0:c0 + w],
                                      lsg[0:rows, 0:w])
                    nc.scalar.activation(out=exs[0:rows, 0:w],
                                         in_=lsg[0:rows, 0:w], func=AF.Exp,
                                         accum_out=acc[0:rows, j:j + 1])
                se = tpa.tile([128, 1], F32, tag="se")
                nc.vector.reduce_sum(out=se[0:rows, :],
                                     in_=acc[0:rows, :]
                                     .rearrange("p (x q) -> p x q", x=1),
                                     axis=mybir.AxisListType.X)
                nc.scalar.activation(out=nlzs[0:rows, nb:nb + 1],
                                     in_=se[0:rows, :], func=AF.Ln)
                nc.vector.tensor_scalar_mul(nlzs[0:rows, nb:nb + 1],
                                            nlzs[0:rows, nb:nb + 1], -1.0)
                # pass B inline: logp = l - logZ, hidden under later steps
                for ch in range(n_c2):
                    c0 = ch * TB2
                    w = min(TB2, V - c0)
                    rd2 = tpb.tile([128, TB2], BF16, tag="rd2")
                    dq = (nc.sync, nc.gpsimd)[ch % 2]
                    dq.dma_start(rd2[0:rows, 0:w],
                                 lstage_d[nb, 0:rows, c0:c0 + w])
                    lp = tpb.tile([128, TB2], F32, tag="lp")
                    nc.vector.tensor_scalar_add(lp[0:rows, 0:w],
                                                rd2[0:rows, 0:w],
                                                nlzs[0:rows, nb:nb + 1])
                    dq2 = (nc.gpsimd, nc.sync)[ch % 2]
                    dq2.dma_start(out_lp.ap()[nb, 0:rows, c0:c0 + w],
                                  lp[0:rows, 0:w])

        ph1.close()

    nc.compile()
    return nc


_NC_CACHE = {}


def _get_program(t_steps=TS, **kw):
    key = (t_steps, tuple(sorted(kw.items())))
    if key not in _NC_CACHE:
        _NC_CACHE[key] = build_program(t_steps, **kw)
    return _NC_CACHE[key]


def make_in_maps(inputs, t_steps=TS):
    bf = ml_dtypes.bfloat16
    f8 = ml_dtypes.float8_e4m3
    enc = np.asarray(inputs["encoder_outputs"], np.float32)
    ehid = np.asarray(inputs["encoder_hidden"], np.float32)
    targets = np.asarray(inputs["targets"])
    embW = np.asarray(inputs["embed_W"], np.float32)
    aW1 = np.asarray(inputs["att_W1"], np.float32)
    aW2 = np.asarray(inputs["att_W2"], np.float32)
    aW3 = np.asarray(inputs["att_W3"], np.float32)
    av = np.asarray(inputs["att_v"], np.float32)
    gWih = np.asarray(inputs["gru_Wih"], np.float32)
    gWhh = np.asarray(inputs["gru_Whh"], np.float32)
    oW = np.asarray(inputs["out_W"], np.float32)
    ab1 = np.asarray(inputs["att_b1"], np.float32)
    ab2 = np.asarray(inputs["att_b2"], np.float32)
    ab3 = np.asarray(inputs["att_b3"], np.float32)
    gbih = np.asarray(inputs["gru_bih"], np.float32)
    gbhh = np.asarray(inputs["gru_bhh"], np.float32)
    ob = np.asarray(inputs["out_b"], np.float32)
    flags = dict(
        use_att_bias=bool(np.abs(ab1).max() or np.abs(ab2).max()
                          or np.abs(ab3).max()),
        use_gru_bias=bool(np.abs(gbih).max() or np.abs(gbhh).max()),
        use_out_bias=bool(np.abs(ob).max()),
    )

    def chunkT(w, dt):  # (out,in) -> [HK, 128, out]
        return np.ascontiguousarray(w.T.astype(dt)).reshape(HK, 128, w.shape[0])

    sel = (np.arange(128)[:, None] % BL == np.arange(BL)[None, :])
    shared = {
        "w1eT": chunkT(aW1[:, :H], bf),
        "w1hT": chunkT(aW1[:, H:], bf),
        "w2T8": chunkT(aW2, f8),
        "w3T8": chunkT(aW3, f8),
        "vT8": np.ascontiguousarray(av[0].astype(f8)).reshape(HK, 128, 1),
        "whhT": chunkT(gWhh, bf),
        "wihcT": chunkT(gWih[:, WORD:], bf),
        "wiheT": chunkT(gWih[:, :WORD], bf),
        "outWT8": chunkT(oW, f8),
        "sel": sel.astype(np.float32),
    }
    if flags["use_att_bias"]:
        shared["b1"] = ab1.reshape(1, H)
        shared["b2"] = np.ascontiguousarray(ab2.reshape(HK, 128).T)
        shared["b3"] = np.ascontiguousarray(ab3.reshape(HK, 128).T)
    if flags["use_gru_bias"]:
        shared["bih"] = gbih.reshape(1, G3)
        bsum = (gbih + gbhh).reshape(GM, 128)
        shared["brz"] = np.ascontiguousarray(bsum[0:8].T)
        shared["bihn"] = np.ascontiguousarray(gbih.reshape(GM, 128)[8:12].T)
        shared["bhhn"] = np.ascontiguousarray(gbhh.reshape(GM, 128)[8:12].T)
    if flags["use_out_bias"]:
        shared["outb"] = ob.reshape(1, V).astype(bf)

    in_maps = []
    for c in range(NC):
        b0 = c * BL
        # l-major rows: row = l*BL + b
        enc_l = np.ascontiguousarray(
            enc[b0:b0 + BL].transpose(1, 0, 2).reshape(NR, H))
        enc_aug = np.concatenate(
            [enc_l, np.ones((NR, 1), np.float32)], axis=1).astype(bf)
        # embeddings: host gather, t-major cols
        idx = targets[b0:b0 + BL, :t_steps].T.astype(np.int64)  # [TS, BL]
        embg = embW[idx.ravel()]                                # [TB, WORD]
        m = dict(shared)
        m["enc_sb"] = np.ascontiguousarray(enc_aug.reshape(RK, 128, H + 1))
        m["encT"] = np.ascontiguousarray(enc_l.T.astype(bf)).reshape(HK, 128, NR)
        m["embT"] = np.ascontiguousarray(embg.T.astype(bf)).reshape(
            HK, 128, t_steps * BL)
        m["hidT0"] = np.ascontiguousarray(
            ehid[0, b0:b0 + BL].T).reshape(HK, 128, BL)
        in_maps.append(m)
    return in_maps, flags


def blocks_of(t_steps):
    blocks, t0 = [], 0
    while t0 < t_steps:
        blocks.append((t0, min(BS, t_steps - t0)))
        t0 += BS
    return blocks


def unshard_out(arrs, t_steps=TS):
    """[NB, 128, V] per core, rows=(tt, b) -> full [B, t_steps, V]."""
    outs = []
    for arr in arrs:
        out = np.empty((BL, t_steps, V), np.float32)
        for nb, (t0, bs) in enumerate(blocks_of(t_steps)):
            out[:, t0:t0 + bs, :] = (
                arr[nb, :bs * BL].reshape(bs, BL, V).transpose(1, 0, 2))
        outs.append(out)
    return np.concatenate(outs, axis=0)


def run(inputs, trace=False, **trace_kw):
    from concourse import bass_utils
    in_maps, flags = make_in_maps(inputs)
    nc = _get_program(**flags)
    res = bass_utils.run_bass_kernel_spmd(nc, in_maps, core_ids=list(range(NC)),
                                          trace=trace, **trace_kw)
    out = unshard_out([res.results[c]["out_lp"] for c in range(NC)])
    return out, res


def kernel(**inputs):
    return run(inputs)[0]


# revision 3
# speedup vs baseline: 2.2992x; 2.2992x over previous
"""DecoderRNN Trainium2 kernel v2 (8 NeuronCores, zero collectives).

Sharding: pure data-parallel over batch (16 rows/core). Each core runs the
full attention+GRU recurrence in transposed layout ([feature-part, batch-free])
and the FULL-vocab output projection + log-softmax locally (no AllGather /
AllReduce). fp8e4m3 DoubleRow matmuls for the attention MLP and the output
projection; ACT engine runs tanh-only during the recurrence (sigmoid via
tanh(x/2), softmax exp via poly on GpSimd) to avoid activation-table reloads.

Self-contained: hardcodes all shapes from the problem spec.
"""
from contextlib import ExitStack

import numpy as np
import ml_dtypes

import concourse.bacc as bacc
import concourse.bass as bass
import concourse.tile as tile
from concourse import mybir
from concourse.bass import AP
from concourse.masks import make_identity

F32 = mybir.dt.float32
BF16 = mybir.dt.bfloat16
F8 = mybir.dt.float8e4
I32 = mybir.dt.int32
AF = mybir.ActivationFunctionType
DR = mybir.MatmulPerfMode.DoubleRow

# problem constants
B, L, H, V, WORD, T = 128, 64, 512, 32000, 512, 32
NC = 8
BL = B // NC          # 16 batch rows per core
NR = BL * L           # 1024 attention rows (l-major: row = l*BL + b)
RK = NR // 128        # 8 row chunks
HK = H // 128         # 4 h chunks
G3 = 3 * H            # 1536
GM = G3 // 128        # 12 gate chunks
TS = T - 1            # 31 steps
TB = TS * BL          # 496 (t-major cols: col = t*BL + b)
BS = 8                # steps per phase-2 block
VT = 512              # vocab tile
TCH = 500             # pass-A exp chunk cols

EXP_C = [1.0, 1.0, 0.5, 1.0 / 6, 1.0 / 24, 1.0 / 120, 1.0 / 720]


def _mm(nc, out, lhsT, rhs, start, stop, pm=None):
    nc.tensor.matmul(out, lhsT, rhs, start=start, stop=stop, perf_mode=pm)


def _bcast(ap, dims):
    """Rebuild AP with explicit [stride, count] dim list."""
    return AP(tensor=ap.tensor, offset=ap.offset, ap=dims)


def build_program(t_steps=TS, use_att_bias=False, use_gru_bias=False,
                  use_out_bias=False):
    nc = bacc.Bacc("TRN2", target_bir_lowering=False, debug=False,
                   num_devices=NC)
    # phase-2 blocks
    blocks = []
    t0 = 0
    while t0 < t_steps:
        blocks.append((t0, min(BS, t_steps - t0)))
        t0 += BS
    NB = len(blocks)
    n_vt = (V + VT - 1) // VT          # 63 (62x512 + 256)
    n_ch = (V + TCH - 1) // TCH        # 32
    tb = t_steps * BL

    def din(name, shape, dt=F32):
        return nc.dram_tensor(name, shape, dt, kind="ExternalInput")

    enc_sb_in = din("enc_sb", [RK, 128, H + 1], BF16)
    encT_in = din("encT", [HK, 128, NR], BF16)
    w1eT_in = din("w1eT", [HK, 128, H], BF16)
    w1hT_in = din("w1hT", [HK, 128, H], F8)
    w2T8_in = din("w2T8", [HK, 128, H], F8)
    w3T8_in = din("w3T8", [HK, 128, H], F8)
    vT8_in = din("vT8", [HK, 128, 1], F8)
    whhT_in = din("whhT", [HK, 128, G3], F8)
    wihcT_in = din("wihcT", [HK, 128, G3], F8)
    wiheT_in = din("wiheT", [HK, 128, G3], BF16)
    embT_in = din("embT", [HK, 128, tb], BF16)
    outWT8_in = din("outWT8", [HK, 128, V], F8)
    sel_in = din("sel", [128, BL])
    hidT0_in = din("hidT0", [HK, 128, BL])
    if use_att_bias:
        b1_in = din("b1", [1, H])
        b2_in = din("b2", [128, HK])
        b3_in = din("b3", [128, HK])
    if use_gru_bias:
        bih_in = din("bih", [1, G3])
        brz_in = din("brz", [128, 8])      # (bih+bhh) r,z parts, T-chunked
        bihn_in = din("bihn", [128, 4])
        bhhn_in = din("bhhn", [128, 4])
    if use_out_bias:
        outb_in = din("outb", [1, V], BF16)
    out_lp = nc.dram_tensor("out_lp", [NB, 128, V], F32,
                            kind="ExternalOutput")

    with tile.TileContext(nc) as tc, ExitStack() as top:
        dram = top.enter_context(tc.tile_pool(name="dram", bufs=1, space="DRAM"))
        giemb_d = dram.tile([GM, 128, tb], BF16)
        lstage_d = dram.tile([NB, 128, V], BF16)

        per = top.enter_context(tc.tile_pool(name="per", bufs=1))
        ident = per.tile([16, 16], F32)
        make_identity(nc, ident[:])
        sel_sb = per.tile([128, BL], F32)
        nc.sync.dma_start(sel_sb[:], sel_in.ap())
        enc_sb = per.tile([128, RK, H + 1], BF16)
        nc.sync.dma_start(enc_sb[:], enc_sb_in.ap().rearrange("k p h -> p k h"))
        w1hT_sb = per.tile([128, HK, H], F8)
        nc.sync.dma_start(w1hT_sb[:], w1hT_in.ap().rearrange("k p h -> p k h"))
        w2T8_sb = per.tile([128, HK, H], F8)
        nc.sync.dma_start(w2T8_sb[:], w2T8_in.ap().rearrange("k p h -> p k h"))
        w3T8_sb = per.tile([128, HK, H], F8)
        nc.sync.dma_start(w3T8_sb[:], w3T8_in.ap().rearrange("k p h -> p k h"))
        vT8_sb = per.tile([128, HK], F8)
        nc.sync.dma_start(vT8_sb[:], vT8_in.ap().rearrange("k p one -> p (k one)"))
        whhT_sb = per.tile([128, HK, G3], F8)
        nc.sync.dma_start(whhT_sb[:], whhT_in.ap().rearrange("k p g -> p k g"))
        wihcT_sb = per.tile([128, HK, G3], F8)
        nc.sync.dma_start(wihcT_sb[:], wihcT_in.ap().rearrange("k p g -> p k g"))
        encprojT = per.tile([128, HK, NR], BF16)
        if use_att_bias:
            b2_sb = per.tile([128, HK], F32)
            nc.sync.dma_start(b2_sb[:], b2_in.ap())
            b3_sb = per.tile([128, HK], F32)
            nc.sync.dma_start(b3_sb[:], b3_in.ap())
        if use_gru_bias:
            brz_sb = per.tile([128, 8], F32)
            nc.sync.dma_start(brz_sb[:], brz_in.ap())
            bihn_sb = per.tile([128, 4], F32)
            nc.sync.dma_start(bihn_sb[:], bihn_in.ap())
            bhhn_sb = per.tile([128, 4], F32)
            nc.sync.dma_start(bhhn_sb[:], bhhn_in.ap())

        # ---------------- phase 0: encproj + gi_emb precompute --------------
        with ExitStack() as ph0:
            p0 = ph0.enter_context(tc.tile_pool(name="p0", bufs=1))
            p0psum = ph0.enter_context(tc.tile_pool(name="p0ps", bufs=2,
                                                    space="PSUM"))
            encT_sb = p0.tile([128, HK, NR], BF16)
            nc.sync.dma_start(encT_sb[:], encT_in.ap().rearrange("k p r -> p k r"))
            w1eT_sb = p0.tile([128, HK, H], BF16)
            nc.sync.dma_start(w1eT_sb[:], w1eT_in.ap().rearrange("k p h -> p k h"))
            wiheT_sb = p0.tile([128, HK, G3], BF16)
            nc.sync.dma_start(wiheT_sb[:], wiheT_in.ap().rearrange("k p g -> p k g"))
            embT_sb = p0.tile([128, HK, tb], BF16)
            nc.sync.dma_start(embT_sb[:], embT_in.ap().rearrange("k p c -> p k c"))
            if use_att_bias:
                b1_sb = p0.tile([1, H], F32)
                nc.sync.dma_start(b1_sb[:], b1_in.ap())
            if use_gru_bias:
                bih_sb = p0.tile([1, G3], F32)
                nc.sync.dma_start(bih_sb[:], bih_in.ap())
            if use_att_bias or use_gru_bias:
                ones_nr = p0.tile([1, 512], F32)
                nc.gpsimd.memset(ones_nr[:], 1.0)

            # encprojT[p=h'%128, m, row] = W1e @ enc^T (+ b1)
            for m in range(HK):
                for half in range(2):
                    sl = slice(half * 512, (half + 1) * 512)
                    pep = p0psum.tile([128, 512], F32, tag="p0ps")
                    for k in range(HK):
                        _mm(nc, pep[:], w1eT_sb[:, k, m * 128:(m + 1) * 128],
                            encT_sb[:, k, sl], k == 0,
                            (not use_att_bias) and k == HK - 1)
                    if use_att_bias:
                        _mm(nc, pep[:], b1_sb[:, m * 128:(m + 1) * 128],
                            ones_nr[:], False, True)
                    nc.scalar.activation(out=encprojT[:, m, sl], in_=pep[:],
                                         func=AF.Copy)
            # gi_embT[p=g%128, m, col=(t,b)] = Wih_e @ emb^T (+ bih)
            p0s = ph0.enter_context(tc.tile_pool(name="p0s", bufs=2))
            for m in range(GM):
                pge = p0psum.tile([128, tb], F32, tag="p0ps")
                for k in range(HK):
                    _mm(nc, pge[:], wiheT_sb[:, k, m * 128:(m + 1) * 128],
                        embT_sb[:, k, :], k == 0,
                        (not use_gru_bias) and k == HK - 1)
                if use_gru_bias:
                    _mm(nc, pge[:], bih_sb[:, m * 128:(m + 1) * 128],
                        ones_nr[:, 0:tb], False, True)
                ge_sb = p0s.tile([128, tb], BF16, tag="ge_sb")
                nc.scalar.activation(out=ge_sb[:], in_=pge[:], func=AF.Copy)
                nc.sync.dma_start(giemb_d[m], ge_sb[:])

        # big fp8 weight load after phase-0 SBUF is released (spread queues)
        outWT8_sb = per.tile([128, HK, V], F8)
        for k in range(HK):
            nc.gpsimd.dma_start(outWT8_sb[:, k, :], outWT8_in.ap()[k])
        nlzs = per.tile([128, NB], F32)
        if use_out_bias:
            outb_sb = per.tile([1, V], BF16)
            nc.sync.dma_start(outb_sb[:], outb_in.ap())
            onesb = per.tile([1, 128], BF16)
            nc.gpsimd.memset(onesb[:], 1.0)

        # ---------------- phase 1 + overlapped phase 2 ----------------------
        ph1 = top.enter_context(ExitStack())
        hidp = ph1.enter_context(tc.tile_pool(name="hidp", bufs=2))
        gw = ph1.enter_context(tc.tile_pool(name="gw", bufs=1))
        wk = ph1.enter_context(tc.tile_pool(name="wk", bufs=2))
        gep = ph1.enter_context(tc.tile_pool(name="gep", bufs=2))
        htp = ph1.enter_context(tc.tile_pool(name="htp", bufs=2))
        lst = ph1.enter_context(tc.tile_pool(name="lst", bufs=2))
        tpa = ph1.enter_context(tc.tile_pool(name="tpa", bufs=2))
        tpb = ph1.enter_context(tc.tile_pool(name="tpb", bufs=2))
        TB2 = 500
        n_c2 = (V + TB2 - 1) // TB2
        # PSUM: pd 2x2 + pp 1 + pg 1 (gh/gi/e/Z) + pm 1 + pcu 1 = 8 banks
        pd = ph1.enter_context(tc.tile_pool(name="pd", bufs=2, space="PSUM"))
        pp = ph1.enter_context(tc.tile_pool(name="pp", bufs=1, space="PSUM"))
        pgp = ph1.enter_context(tc.tile_pool(name="pg", bufs=1, space="PSUM"))
        pmp = ph1.enter_context(tc.tile_pool(name="pm", bufs=1, space="PSUM"))

        hidf = hidp.tile([128, HK, BL], F32, tag="hidf")
        nc.sync.dma_start(hidf[:], hidT0_in.ap().rearrange("k p b -> p k b"))
        hidT = hidp.tile([128, HK, BL], F8, tag="hidT")
        nc.vector.tensor_copy(hidT[:], hidf[:])

        t2blk = {}
        for i, (bt0, bs) in enumerate(blocks):
            for tt in range(bs):
                t2blk[bt0 + tt] = (i, tt)
        hT_blk = None
        for t in range(t_steps):
            nb, tt = t2blk[t]
            if tt == 0:
                hT_blk = htp.tile([128, HK, BS, BL], F8, tag="hT")

            ge = gep.tile([128, GM, BL], BF16, tag="ge")
            nc.sync.dma_start(
                ge[:], giemb_d[:, :, t * BL:(t + 1) * BL]
                .rearrange("m p b -> p m b"))

            # hidproj (pm slot)
            php = pmp.tile([128, HK, BL], F32, tag="pm")
            for m in range(HK):
                for kk in range(2):
                    _mm(nc, php[:, m, :],
                        w1hT_sb[:, 2 * kk:2 * kk + 2, m * 128:(m + 1) * 128],
                        hidT[:, 2 * kk:2 * kk + 2, :], kk == 0, kk == 1, pm=DR)
            hp_sb = gw.tile([128, HK, BL], BF16, tag="hp_sb")
            nc.scalar.activation(out=hp_sb[:], in_=php[:], func=AF.Copy)

            # psum: [0:8] = gh_rz + gi_rz (merged groups), [8:12] = gh_n,
            # [12:16] = gi_n
            pg = pgp.tile([128, 16, BL], F32, tag="pg")
            for m in range(8, GM):      # gh_n early (PE idle at step start)
                for kk in range(2):
                    _mm(nc, pg[:, m, :],
                        whhT_sb[:, 2 * kk:2 * kk + 2, m * 128:(m + 1) * 128],
                        hidT[:, 2 * kk:2 * kk + 2, :], kk == 0, kk == 1, pm=DR)
            pgh = pg[:, 0:GM, :]

            # attention MLP over row-halves
            a13 = gw.tile([128, HK, NR], F8, tag="a13")   # a1 then a3 in place
            a2T8 = gw.tile([128, HK, NR], F8, tag="a2T8")
            pe = pmp.tile([128, RK], F32, tag="pm")
            e_sb = gw.tile([128, RK], F32, tag="e_sb")
            pt1 = gw.tile([128, RK], F32, tag="pt1")
            mask_sb = gw.tile([128, RK, BL], BF16, tag="mask")
            pcu = pmp.tile([16, 512], F32, tag="pm2")
            for hf in range(2):
                sl = slice(hf * 512, (hf + 1) * 512)
                a1pre = wk.tile([128, HK, 512], BF16, tag="a1pre")
                hb = _bcast(hp_sb[:], [hp_sb[:].ap[0], [BL, HK], [0, 32], [1, BL]])
                nc.vector.tensor_add(a1pre[:], encprojT[:, :, sl], hb)
                nc.scalar.activation(out=a13[:, :, sl], in_=a1pre[:],
                                     func=AF.Tanh)
                for (src, dst, wT, bsb) in ((a13, a2T8, w2T8_sb, "b2"),
                                            (a2T8, a13, w3T8_sb, "b3")):
                    for mp in range(2):      # m-pairs -> one 2-bank psum tile
                        pdt = pd.tile([128, 2, 512], F32, tag="pd")
                        for mi in range(2):
                            m = 2 * mp + mi
                            for kk in range(2):
                                _mm(nc, pdt[:, mi, :],
                                    wT[:, 2 * kk:2 * kk + 2,
                                       m * 128:(m + 1) * 128],
                                    src[:, 2 * kk:2 * kk + 2, sl],
                                    kk == 0, kk == 1, pm=DR)
                        if use_att_bias:
                            bb = b2_sb if bsb == "b2" else b3_sb
                            for mi in range(2):
                                m = 2 * mp + mi
                                nc.scalar.activation(out=dst[:, m, sl],
                                                     in_=pdt[:, mi, :],
                                                     func=AF.Tanh,
                                                     bias=bb[:, m:m + 1])
                        else:
                            nc.scalar.activation(
                                out=dst[:, 2 * mp:2 * mp + 2, sl],
                                in_=pdt[:], func=AF.Tanh)
                # e rows for this half (a13 now holds a3)
                rsl = slice(hf * 4, hf * 4 + 4)
                for r in range(hf * 4, hf * 4 + 4):
                    for k in range(HK):
                        _mm(nc, pe[:, r:r + 1],
                            a13[:, k, r * 128:(r + 1) * 128],
                            vT8_sb[:, k:k + 1], k == 0, k == HK - 1)
                # exp(e) poly-6 on DVE (keeps ACT tanh-only)
                nc.vector.tensor_copy(e_sb[:, rsl], pe[:, rsl])
                nc.gpsimd.tensor_scalar(pt1[:, rsl], e_sb[:, rsl], EXP_C[6],
                                        EXP_C[5], mybir.AluOpType.mult,
                                        mybir.AluOpType.add)
                for dg in (4, 3, 2, 1, 0):
                    nc.gpsimd.tensor_mul(pt1[:, rsl], pt1[:, rsl], e_sb[:, rsl])
                    nc.gpsimd.tensor_scalar_add(pt1[:, rsl], pt1[:, rsl],
                                                EXP_C[dg])
                ea = _bcast(pt1[:, rsl], [pt1[:].ap[0], [1, 4], [0, BL]])
                sa = _bcast(sel_sb[:], [sel_sb[:].ap[0], [0, 4], [1, BL]])
                nc.vector.tensor_mul(mask_sb[:, rsl, :], ea, sa)
                for k in range(hf * 4, hf * 4 + 4):
                    _mm(nc, pcu[:], mask_sb[:, k, :], enc_sb[:, k, 0:H],
                        k == 0, k == RK - 1)

            pz = pmp.tile([16, 1], F32, tag="pm")
            for k in range(RK):
                _mm(nc, pz[:], mask_sb[:, k, :], enc_sb[:, k, H:H + 1],
                    k == 0, k == RK - 1)
            ctxu_sb = gw.tile([16, 512], BF16, tag="ctxu")
            nc.scalar.activation(out=ctxu_sb[:], in_=pcu[:], func=AF.Copy)
            rcpZ = gw.tile([16, 1], F32, tag="rcpZ")
            nc.vector.reciprocal(rcpZ[:], pz[:])
            diag = gw.tile([16, 16], BF16, tag="diag")
            nc.vector.tensor_scalar_mul(diag[:], ident[:], rcpZ[:])
            pct = pmp.tile([128, HK, BL], F32, tag="pm")
            for m in range(HK):
                _mm(nc, pct[:, m, :], ctxu_sb[:, m * 128:(m + 1) * 128],
                    diag[:], True, True)
            ctxT_sb = gw.tile([128, HK, BL], F8, tag="ctxT")
            nc.vector.tensor_copy(ctxT_sb[:], pct[:])

            # gi: rz parts continue gh_rz accumulation groups; n part separate
            for m in range(8):
                for kk in range(2):
                    _mm(nc, pg[:, m, :],
                        whhT_sb[:, 2 * kk:2 * kk + 2, m * 128:(m + 1) * 128],
                        hidT[:, 2 * kk:2 * kk + 2, :], kk == 0, False, pm=DR)
                for kk in range(2):
                    _mm(nc, pg[:, m, :],
                        wihcT_sb[:, 2 * kk:2 * kk + 2, m * 128:(m + 1) * 128],
                        ctxT_sb[:, 2 * kk:2 * kk + 2, :], False, kk == 1, pm=DR)
            for m in range(4):
                for kk in range(2):
                    _mm(nc, pg[:, 12 + m, :],
                        wihcT_sb[:, 2 * kk:2 * kk + 2,
                                 (8 + m) * 128:(9 + m) * 128],
                        ctxT_sb[:, 2 * kk:2 * kk + 2, :], kk == 0, kk == 1,
                        pm=DR)
            pgi = _bcast(pg[:, 12:16, :], None)  # placeholder, unused
            # gates: s = tanh(x/2); sigmoid(x) = (s+1)/2
            rzs = gw.tile([128, 8, BL], F32, tag="rzs")
            nc.vector.tensor_add(rzs[:], pg[:, 0:8, :], ge[:, 0:8, :])
            if use_gru_bias:
                for m in range(8):
                    nc.vector.tensor_scalar_add(rzs[:, m, :], rzs[:, m, :],
                                                brz_sb[:, m:m + 1])
            srz = gw.tile([128, 8, BL], F32, tag="srz")
            nc.scalar.activation(out=srz[:], in_=rzs[:], func=AF.Tanh, scale=0.5)
            u1 = gw.tile([128, 4, BL], F32, tag="u1")
            nc.vector.tensor_add(u1[:], pgi[:, 8:12, :], ge[:, 8:12, :])
            if use_gru_bias:
                for m in range(4):
                    nc.vector.tensor_scalar_add(u1[:, m, :], u1[:, m, :],
                                                bihn_sb[:, m:m + 1])
            u2 = gw.tile([128, 4, BL], F32, tag="u2")
            if use_gru_bias:
                ghnb = gw.tile([128, 4, BL], F32, tag="ghnb")
                for m in range(4):
                    nc.vector.tensor_scalar_add(ghnb[:, m, :], pgh[:, 8 + m, :],
                                                bhhn_sb[:, m:m + 1])
                nc.vector.tensor_mul(u2[:], srz[:, 0:4, :], ghnb[:])
                nc.vector.tensor_add(u2[:], u2[:], ghnb[:])
            else:
                nc.vector.tensor_mul(u2[:], srz[:, 0:4, :], pgh[:, 8:12, :])
                nc.vector.tensor_add(u2[:], u2[:], pgh[:, 8:12, :])
            nc.vector.tensor_scalar_mul(u2[:], u2[:], 0.5)
            nc.vector.tensor_add(u1[:], u1[:], u2[:])
            n_t = gw.tile([128, 4, BL], F32, tag="n_t")
            nc.scalar.activation(out=n_t[:], in_=u1[:], func=AF.Tanh)
            d_t = gw.tile([128, 4, BL], F32, tag="d_t")
            nc.vector.tensor_sub(d_t[:], hidf[:], n_t[:])
            v1 = gw.tile([128, 4, BL], F32, tag="v1")
            nc.vector.tensor_mul(v1[:], srz[:, 4:8, :], d_t[:])
            nc.vector.tensor_add(v1[:], v1[:], d_t[:])
            nc.vector.tensor_scalar_mul(v1[:], v1[:], 0.5)
            hidf = hidp.tile([128, HK, BL], F32, tag="hidf")
            nc.vector.tensor_add(hidf[:], n_t[:], v1[:])
            hidT = hidp.tile([128, HK, BL], F8, tag="hidT")
            nc.vector.tensor_copy(hidT[:], hidf[:])
            nc.vector.tensor_copy(hT_blk[:, :, tt, :], hidf[:])

            # -------- phase 2 pass A for completed block ----------
            if tt == blocks[nb][1] - 1:
                bs = tt + 1
                rows = bs * BL
                acc = tpa.tile([128, n_vt], F32, tag="acc")
                exs = tpa.tile([128, VT], BF16, tag="exs")
                for j in range(n_vt):
                    c0 = j * VT
                    w = min(VT, V - c0)
                    ppt = pp.tile([128, VT], F32, tag="pp")
                    for kk in range(2):
                        _mm(nc, ppt[0:rows, 0:w],
                            hT_blk[:, 2 * kk:2 * kk + 2, 0:bs, :],
                            outWT8_sb[:, 2 * kk:2 * kk + 2, c0:c0 + w],
                            kk == 0, kk == 1 and not use_out_bias, pm=DR)
                    if use_out_bias:
                        _mm(nc, ppt[0:rows, 0:w], onesb[:, 0:rows],
                            outb_sb[:, c0:c0 + w], False, True)
                    lsg = lst.tile([128, VT], BF16, tag="lsg")
                    nc.vector.tensor_copy(lsg[0:rows, 0:w], ppt[0:rows, 0:w])
                    nc.sync.dma_start(lstage_d[nb, 0:rows, c0:c0 + w],
                                      lsg[0:rows, 0:w])
                    nc.scalar.activation(out=exs[0:rows, 0:w],
                                         in_=lsg[0:rows, 0:w], func=AF.Exp,
                                         accum_out=acc[0:rows, j:j + 1])
                se = tpa.tile([128, 1], F32, tag="se")
                nc.vector.reduce_sum(out=se[0:rows, :],
                                     in_=acc[0:rows, :]
                                     .rearrange("p (x q) -> p x q", x=1),
                                     axis=mybir.AxisListType.X)
                nc.scalar.activation(out=nlzs[0:rows, nb:nb + 1],
                                     in_=se[0:rows, :], func=AF.Ln)
                nc.vector.tensor_scalar_mul(nlzs[0:rows, nb:nb + 1],
                                            nlzs[0:rows, nb:nb + 1], -1.0)
                # pass B inline: logp = l - logZ, hidden under later steps
                for ch in range(n_c2):
                    c0 = ch * TB2
                    w = min(TB2, V - c0)
                    rd2 = tpb.tile([128, TB2], BF16, tag="rd2")
                    dq = (nc.sync, nc.gpsimd)[ch % 2]
                    dq.dma_start(rd2[0:rows, 0:w],
                                 lstage_d[nb, 0:rows, c0:c0 + w])
                    lp = tpb.tile([128, TB2], F32, tag="lp")
                    nc.vector.tensor_scalar_add(lp[0:rows, 0:w],
                                                rd2[0:rows, 0:w],
                                                nlzs[0:rows, nb:nb + 1])
                    dq2 = (nc.gpsimd, nc.sync)[ch % 2]
                    dq2.dma_start(out_lp.ap()[nb, 0:rows, c0:c0 + w],
                                  lp[0:rows, 0:w])

        ph1.close()

    nc.compile()
    return nc


_NC_CACHE = {}


def _get_program(t_steps=TS, **kw):
    key = (t_steps, tuple(sorted(kw.items())))
    if key not in _NC_CACHE:
        _NC_CACHE[key] = build_program(t_steps, **kw)
    return _NC_CACHE[key]


def make_in_maps(inputs, t_steps=TS):
    bf = ml_dtypes.bfloat16
    f8 = ml_dtypes.float8_e4m3
    enc = np.asarray(inputs["encoder_outputs"], np.float32)
    ehid = np.asarray(inputs["encoder_hidden"], np.float32)
    targets = np.asarray(inputs["targets"])
    embW = np.asarray(inputs["embed_W"], np.float32)
    aW1 = np.asarray(inputs["att_W1"], np.float32)
    aW2 = np.asarray(inputs["att_W2"], np.float32)
    aW3 = np.asarray(inputs["att_W3"], np.float32)
    av = np.asarray(inputs["att_v"], np.float32)
    gWih = np.asarray(inputs["gru_Wih"], np.float32)
    gWhh = np.asarray(inputs["gru_Whh"], np.float32)
    oW = np.asarray(inputs["out_W"], np.float32)
    ab1 = np.asarray(inputs["att_b1"], np.float32)
    ab2 = np.asarray(inputs["att_b2"], np.float32)
    ab3 = np.asarray(inputs["att_b3"], np.float32)
    gbih = np.asarray(inputs["gru_bih"], np.float32)
    gbhh = np.asarray(inputs["gru_bhh"], np.float32)
    ob = np.asarray(inputs["out_b"], np.float32)
    flags = dict(
        use_att_bias=bool(np.abs(ab1).max() or np.abs(ab2).max()
                          or np.abs(ab3).max()),
        use_gru_bias=bool(np.abs(gbih).max() or np.abs(gbhh).max()),
        use_out_bias=bool(np.abs(ob).max()),
    )

    def chunkT(w, dt):  # (out,in) -> [HK, 128, out]
        return np.ascontiguousarray(w.T.astype(dt)).reshape(HK, 128, w.shape[0])

    sel = (np.arange(128)[:, None] % BL == np.arange(BL)[None, :])
    shared = {
        "w1eT": chunkT(aW1[:, :H], bf),
        "w1hT": chunkT(aW1[:, H:], f8),
        "w2T8": chunkT(aW2, f8),
        "w3T8": chunkT(aW3, f8),
        "vT8": np.ascontiguousarray(av[0].astype(f8)).reshape(HK, 128, 1),
        "whhT": chunkT(gWhh, f8),
        "wihcT": chunkT(gWih[:, WORD:], f8),
        "wiheT": chunkT(gWih[:, :WORD], bf),
        "outWT8": chunkT(oW, f8),
        "sel": sel.astype(np.float32),
    }
    if flags["use_att_bias"]:
        shared["b1"] = ab1.reshape(1, H)
        shared["b2"] = np.ascontiguousarray(ab2.reshape(HK, 128).T)
        shared["b3"] = np.ascontiguousarray(ab3.reshape(HK, 128).T)
    if flags["use_gru_bias"]:
        shared["bih"] = gbih.reshape(1, G3)
        bsum = (gbih + gbhh).reshape(GM, 128)
        shared["brz"] = np.ascontiguousarray(bsum[0:8].T)
        shared["bihn"] = np.ascontiguousarray(gbih.reshape(GM, 128)[8:12].T)
        shared["bhhn"] = np.ascontiguousarray(gbhh.reshape(GM, 128)[8:12].T)
    if flags["use_out_bias"]:
        shared["outb"] = ob.reshape(1, V).astype(bf)

    in_maps = []
    for c in range(NC):
        b0 = c * BL
        # l-major rows: row = l*BL + b
        enc_l = np.ascontiguousarray(
            enc[b0:b0 + BL].transpose(1, 0, 2).reshape(NR, H))
        enc_aug = np.concatenate(
            [enc_l, np.ones((NR, 1), np.float32)], axis=1).astype(bf)
        # embeddings: host gather, t-major cols
        idx = targets[b0:b0 + BL, :t_steps].T.astype(np.int64)  # [TS, BL]
        embg = embW[idx.ravel()]                                # [TB, WORD]
        m = dict(shared)
        m["enc_sb"] = np.ascontiguousarray(enc_aug.reshape(RK, 128, H + 1))
        m["encT"] = np.ascontiguousarray(enc_l.T.astype(bf)).reshape(HK, 128, NR)
        m["embT"] = np.ascontiguousarray(embg.T.astype(bf)).reshape(
            HK, 128, t_steps * BL)
        m["hidT0"] = np.ascontiguousarray(
            ehid[0, b0:b0 + BL].T).reshape(HK, 128, BL)
        in_maps.append(m)
    return in_maps, flags


def blocks_of(t_steps):
    blocks, t0 = [], 0
    while t0 < t_steps:
        blocks.append((t0, min(BS, t_steps - t0)))
        t0 += BS
    return blocks


def unshard_out(arrs, t_steps=TS):
    """[NB, 128, V] per core, rows=(tt, b) -> full [B, t_steps, V]."""
    outs = []
    for arr in arrs:
        out = np.empty((BL, t_steps, V), np.float32)
        for nb, (t0, bs) in enumerate(blocks_of(t_steps)):
            out[:, t0:t0 + bs, :] = (
                arr[nb, :bs * BL].reshape(bs, BL, V).transpose(1, 0, 2))
        outs.append(out)
    return np.concatenate(outs, axis=0)


def run(inputs, trace=False, **trace_kw):
    from concourse import bass_utils
    in_maps, flags = make_in_maps(inputs)
    nc = _get_program(**flags)
    res = bass_utils.run_bass_kernel_spmd(nc, in_maps, core_ids=list(range(NC)),
                                          trace=trace, **trace_kw)
    out = unshard_out([res.results[c]["out_lp"] for c in range(NC)])
    return out, res


def kernel(**inputs):
    return run(inputs)[0]
